# revision 2
# baseline (speedup 1.0000x reference)
"""Trainium2 Bass kernel: GQA attention block (QKV proj + RoPE + causal attention + o_proj).

Tensor-parallel over heads across 8 NeuronCores. Each core computes 4 Q heads /
1 KV head and a partial o_proj output; the host sums the 8 partials.

Self-contained: hardcodes problem shapes from the spec
  hidden_states [1, 2048, 4096], w_qkv [4096, 6144], w_o [4096, 4096],
  attention_mask causal tril [1,1,2048,2048], position_ids arange [1, 2048].
"""

import math
import sys

sys.path.insert(0, "/opt/trn_rl_repo")

import numpy as np

# problem shapes
B, S, HID = 1, 2048, 4096
NH, NKV, D = 32, 8, 128
ROPE_THETA = 10000.0
NCORES = 8
P = 128

# per-core tiling config
CFG = dict(S=S, HID=HID, NQH=NH // NCORES, SEQT=512, QT=512, NT=512, MGROUPS=2)

# set by test harness to capture HW exec time via NTFF trace
TRACE = False
LAST_EXEC_NS = None


def build_tile_kernel(tc, ins, out, S, HID, NQH, SEQT, QT, NT, MGROUPS):
    """Emit the per-core attention kernel into TileContext `tc`.

    ins: dict of DRAM APs:
      hsT   [HID, S]    f32  hidden_states transposed (replicated)
      wq    [HID, (NQH+2)*128] f32  w_qkv column slice (Q heads | K | V)
      wo    [NQH*128, HID] f32      w_o row slice
      cosT  [128, S]    f32  rope cos, transposed
      sinT  [128, S]    f32  rope sin, transposed, rows 0..63 negated
      dmask [128, QT//128, QT] f32  causal masks for diagonal tiles
      ones  [128, 128]  f32  all-ones (softmax denominator matmul)
    out: [S, HID] f32 partial output
    """
    import concourse.bass as bass
    from concourse import mybir
    from concourse.masks import make_identity

    nc = tc.nc
    f32, f32r = mybir.dt.float32, mybir.dt.float32r
    Exp = mybir.ActivationFunctionType.Exp

    MT = NQH + 2          # qkv out tiles of 128 cols: NQH q heads, k, v
    assert MT % MGROUPS == 0
    MG = MT // MGROUPS    # m-tiles per group
    KH = HID // P         # contraction chunks for qkv/o projections
    STn = S // SEQT       # seq tiles in qkv phase
    KCn = S // P          # key chunks in attention
    QTn = S // QT         # q tiles in attention
    RD = QT // P          # diagonal offsets per q tile
    NTn = HID // NT       # oproj col tiles
    KD = NQH              # oproj contraction chunks (d-chunks = q heads)
    qk_scale = 1.0 / math.sqrt(D)

    hsT, wq, wo, cosT_d, sinT_d, dmask_d, ones_d = (
        ins["hsT"], ins["wq"], ins["wo"], ins["cosT"], ins["sinT"],
        ins["dmask"], ins["ones"],
    )

    import contextlib

    with contextlib.ExitStack() as ctx:
        keep = ctx.enter_context(tc.tile_pool(name="keep", bufs=1))
        dram = ctx.enter_context(tc.tile_pool(name="dram", bufs=1, space="DRAM"))
        hsp = ctx.enter_context(tc.tile_pool(name="hsp", bufs=3))
        rope = ctx.enter_context(tc.tile_pool(name="rope", bufs=1))
        expp = ctx.enter_context(tc.tile_pool(name="expp", bufs=4))
        outp = ctx.enter_context(tc.tile_pool(name="outp", bufs=4))
        astr = ctx.enter_context(tc.tile_pool(name="astr", bufs=3))
        psum = ctx.enter_context(tc.tile_pool(name="psum", bufs=8, space="PSUM"))

        # persistent stores
        qk_store = [keep.tile([P, S], f32r, tag=f"qk{i}", name=f"qk{i}") for i in range(NQH + 1)]
        v_store = keep.tile([P, KCn, D], f32r, tag="vst")
        dmask = keep.tile([P, RD, QT], f32r, tag="dmask")
        ones = keep.tile([P, P], f32r, tag="ones")
        ident = keep.tile([P, P], f32, tag="ident")
        attn_scr = dram.tile([P, NQH, S], f32r)

        nc.sync.dma_start(out=dmask[:], in_=dmask_d[:].bitcast(f32r))
        nc.sync.dma_start(out=ones[:], in_=ones_d[:].bitcast(f32r))
        make_identity(nc, ident[:])

        # ---- phase 1: QKV projection (transposed out) + RoPE, in M-groups ----
        with tc.tile_pool(name="wqp", bufs=1) as wqp:
            cosT = wqp.tile([P, S], f32, tag="cos")
            sinT = wqp.tile([P, S], f32, tag="sin")
            nc.sync.dma_start(out=cosT[:], in_=cosT_d[:])
            nc.sync.dma_start(out=sinT[:], in_=sinT_d[:])

            wq3 = wq.rearrange("(kc p) m -> p kc m", p=P)  # [P, KH, MT*P]

            for g in range(MGROUPS):
                gm0 = g * MG  # first m-tile of this group
                wq_sb = wqp.tile([P, KH, MG * P], f32r, tag="wq")
                for kc in range(KH):
                    nc.sync.dma_start(
                        out=wq_sb[:, kc, :],
                        in_=wq3[:, kc, gm0 * P:(gm0 + MG) * P].bitcast(f32r),
                    )
                for st in range(STn):
                    cs = slice(st * SEQT, (st + 1) * SEQT)
                    ps_tiles = [
                        psum.tile([P, SEQT], f32, tag="ps", name=f"qkvps{_i}")
                        for _i in range(MG)
                    ]
                    for kc in range(KH):
                        hs_t = hsp.tile([P, SEQT], f32r, tag="hs")
                        nc.sync.dma_start(
                            out=hs_t[:],
                            in_=hsT[kc * P:(kc + 1) * P, cs].bitcast(f32r),
                        )
                        for mi in range(MG):
                            nc.tensor.matmul(
                                ps_tiles[mi][:],
                                wq_sb[:, kc, mi * P:(mi + 1) * P],
                                hs_t[:],
                                start=(kc == 0),
                                stop=(kc == KH - 1),
                            )
                    for mi in range(MG):
                        mt = gm0 + mi
                        if mt < NQH + 1:
                            # rope: out = x*cos + swap_halves(x)*sin'
                            tmp = rope.tile([P, SEQT], f32, tag="tmp")
                            nc.scalar.copy(tmp[:], ps_tiles[mi][:])
                            m1 = rope.tile([P, SEQT], f32, tag="m1")
                            nc.vector.tensor_mul(m1[:], ps_tiles[mi][:], cosT[:, cs])
                            sw = rope.tile([P, SEQT], f32, tag="sw")
                            nc.sync.dma_start(out=sw[0:64, :], in_=tmp[64:128, :])
                            nc.sync.dma_start(out=sw[64:128, :], in_=tmp[0:64, :])
                            m2 = rope.tile([P, SEQT], f32, tag="m2")
                            nc.vector.tensor_mul(m2[:], sw[:], sinT[:, cs])
                            nc.vector.tensor_add(qk_store[mt][:, cs], m1[:], m2[:])
                        else:
                            # V: transpose 128-blocks into [k, d] layout
                            vt = rope.tile([P, SEQT], f32, tag="vt")
                            nc.scalar.copy(vt[:], ps_tiles[mi][:])
                            for j in range(SEQT // P):
                                tp = psum.tile([P, P], f32, tag="ps")
                                nc.tensor.transpose(
                                    tp[:], vt[:, j * P:(j + 1) * P], ident[:]
                                )
                                nc.vector.tensor_copy(
                                    v_store[:, st * (SEQT // P) + j, :], tp[:]
                                )

        # ---- phase 2/3: attention (scoresT -> exp -> PV + denom) and o_proj ----
        with tc.tile_pool(name="wop", bufs=1) as wop:
            wo_sb = wop.tile([P, KD, HID], f32r, tag="wo")
            wo3 = wo.rearrange("(kd p) n -> p kd n", p=P)
            for kd in range(KD):
                nc.sync.dma_start(
                    out=wo_sb[:, kd, :], in_=wo3[:, kd, :].bitcast(f32r)
                )

            for qt in range(QTn):
                qs = slice(qt * QT, (qt + 1) * QT)
                for h in range(NQH):
                    ao = psum.tile([P, QT], f32, tag="ps")
                    dn = psum.tile([P, QT], f32, tag="ps")
                    nkb = RD * qt + RD
                    for kb in range(nkb):
                        sp = psum.tile([P, QT], f32, tag="ps")
                        nc.tensor.matmul(
                            sp[:],
                            qk_store[NQH][:, kb * P:(kb + 1) * P],
                            qk_store[h][:, qs],
                            start=True,
                            stop=True,
                        )
                        ex = expp.tile([P, QT], f32r, tag="ex")
                        nc.scalar.activation(
                            out=ex[:], in_=sp[:], func=Exp, scale=qk_scale
                        )
                        r = kb - RD * qt
                        if r >= 0:
                            nc.vector.tensor_mul(ex[:], ex[:], dmask[:, r, :])
                        nc.tensor.matmul(
                            ao[:], v_store[:, kb, :], ex[:],
                            start=(kb == 0), stop=(kb == nkb - 1),
                        )
                        nc.tensor.matmul(
                            dn[:], ones[:], ex[:],
                            start=(kb == 0), stop=(kb == nkb - 1),
                        )
                    rc = expp.tile([P, QT], f32, tag="rc")
                    nc.vector.reciprocal(rc[:], dn[:])
                    an = expp.tile([P, QT], f32r, tag="an")
                    nc.vector.tensor_mul(an[:], ao[:], rc[:])
                    nc.sync.dma_start(out=attn_scr[:, h, qs], in_=an[:])

                # o_proj for the s-tiles covered by this q tile
                for st in range(qt * (QT // P), (qt + 1) * (QT // P)):
                    ss = slice(st * P, (st + 1) * P)
                    at = astr.tile([P, KD, P], f32r, tag="at")
                    nc.sync.dma_start(out=at[:], in_=attn_scr[:, :, ss])
                    for nt in range(NTn):
                        po = psum.tile([P, NT], f32, tag="ps")
                        for kd in range(KD):
                            nc.tensor.matmul(
                                po[:],
                                at[:, kd, :],
                                wo_sb[:, kd, nt * NT:(nt + 1) * NT],
                                start=(kd == 0),
                                stop=(kd == KD - 1),
                            )
                        ot = outp.tile([P, NT], f32, tag="ot")
                        nc.vector.tensor_copy(ot[:], po[:])
                        nc.sync.dma_start(
                            out=out[ss, nt * NT:(nt + 1) * NT], in_=ot[:]
                        )


def _host_inputs(hidden_states, w_qkv, w_o, position_ids):
    """Shard/transform full inputs into per-core input maps."""
    NQH = CFG["NQH"]
    QT = CFG["QT"]
    hs = np.ascontiguousarray(
        np.asarray(hidden_states, dtype=np.float32).reshape(S, HID).T
    )  # [HID, S]

    wqkv = np.asarray(w_qkv, dtype=np.float32)
    wo_full = np.asarray(w_o, dtype=np.float32)

    pos = np.asarray(position_ids).reshape(-1).astype(np.float32)  # [S]
    inv_freq = (
        1.0 / (ROPE_THETA ** (np.arange(0, D, 2, dtype=np.float32) / D))
    ).astype(np.float32)  # [D/2]
    freqs = pos[:, None] * inv_freq[None, :]  # [S, D/2]
    emb = np.concatenate([freqs, freqs], axis=1)  # [S, D]
    cosT = np.ascontiguousarray(np.cos(emb).T.astype(np.float32))  # [D, S]
    sinT = np.ascontiguousarray(np.sin(emb).T.astype(np.float32))
    sinT[: D // 2] *= -1.0  # fold rotate_half sign into sin

    RD = QT // P
    kk = np.arange(P)[:, None]
    jq = np.arange(QT)[None, :]
    dmask = np.stack(
        [(r * P + kk <= jq).astype(np.float32) for r in range(RD)], axis=1
    )  # [P, RD, QT]
    dmask = np.ascontiguousarray(dmask)
    ones = np.ones((P, P), dtype=np.float32)

    in_maps = []
    for c in range(NCORES):
        qcols = wqkv[:, c * NQH * D:(c + 1) * NQH * D]
        kcols = wqkv[:, NH * D + c * D: NH * D + (c + 1) * D]
        vcols = wqkv[:, (NH + NKV) * D + c * D: (NH + NKV) * D + (c + 1) * D]
        wq_c = np.ascontiguousarray(np.concatenate([qcols, kcols, vcols], axis=1))
        wo_c = np.ascontiguousarray(wo_full[c * NQH * D:(c + 1) * NQH * D, :])
        in_maps.append(
            dict(hsT=hs, wq=wq_c, wo=wo_c, cosT=cosT, sinT=sinT,
                 dmask=dmask, ones=ones)
        )
    return in_maps


_NC_CACHE = []


def _build_nc():
    import concourse.tile as tile
    from concourse import bacc, mybir

    f32 = mybir.dt.float32
    NQH, QT = CFG["NQH"], CFG["QT"]
    nc = bacc.Bacc("TRN2", target_bir_lowering=False, debug=False,
                   num_devices=NCORES)
    ins = {
        "hsT": nc.dram_tensor("hsT", [HID, S], f32, kind="ExternalInput").ap(),
        "wq": nc.dram_tensor("wq", [HID, (NQH + 2) * P], f32,
                             kind="ExternalInput").ap(),
        "wo": nc.dram_tensor("wo", [NQH * P, HID], f32,
                             kind="ExternalInput").ap(),
        "cosT": nc.dram_tensor("cosT", [P, S], f32, kind="ExternalInput").ap(),
        "sinT": nc.dram_tensor("sinT", [P, S], f32, kind="ExternalInput").ap(),
        "dmask": nc.dram_tensor("dmask", [P, QT // P, QT], f32,
                                kind="ExternalInput").ap(),
        "ones": nc.dram_tensor("ones", [P, P], f32, kind="ExternalInput").ap(),
    }
    out = nc.dram_tensor("out", [S, HID], f32, kind="ExternalOutput").ap()
    with tile.TileContext(nc) as tc:
        build_tile_kernel(tc, ins, out, **CFG)
    nc.compile()
    return nc


def kernel(hidden_states, w_qkv, w_o, attention_mask, position_ids):
    global LAST_EXEC_NS
    from concourse.bass_utils import run_bass_kernel_spmd

    if not _NC_CACHE:
        _NC_CACHE.append(_build_nc())
    nc = _NC_CACHE[0]

    in_maps = _host_inputs(hidden_states, w_qkv, w_o, position_ids)
    res = run_bass_kernel_spmd(nc, in_maps, list(range(NCORES)), trace=TRACE)
    LAST_EXEC_NS = res.exec_time_ns

    acc = np.zeros((S, HID), dtype=np.float32)
    for c in range(NCORES):
        acc += res.results[c]["out"]
    return acc.reshape(B, S, HID)


# revision 4
# speedup vs baseline: 1.5316x; 1.5316x over previous
"""Trainium2 Bass kernel: GQA attention block (QKV proj + RoPE + causal attention + o_proj).

Tensor-parallel over heads across 8 NeuronCores. Each core computes 4 Q heads /
1 KV head and a partial o_proj output; the host sums the 8 partials.

Compute: bf16 matmuls (fp32 PSUM accumulate), fp32 softmax/rope elementwise.

Self-contained: hardcodes problem shapes from the spec
  hidden_states [1, 2048, 4096], w_qkv [4096, 6144], w_o [4096, 4096],
  attention_mask causal tril [1,1,2048,2048], position_ids arange [1, 2048].
"""

import math
import sys

sys.path.insert(0, "/opt/trn_rl_repo")

import numpy as np

# problem shapes
B, S, HID = 1, 2048, 4096
NH, NKV, D = 32, 8, 128
ROPE_THETA = 10000.0
NCORES = 8
P = 128

# per-core tiling config
CFG = dict(S=S, HID=HID, NQH=NH // NCORES, SEQT=512, QT=512, NT=512)

# set by test harness to capture HW exec time via NTFF trace
TRACE = False
LAST_EXEC_NS = None


def build_tile_kernel(tc, ins, out, S, HID, NQH, SEQT, QT, NT):
    """Emit the per-core attention kernel into TileContext `tc`.

    ins: dict of DRAM APs:
      hsT   [HID, S]    bf16  hidden_states transposed (replicated)
      wq    [HID, (NQH+2)*128] bf16  w_qkv column slice (Q heads | K | V)
      wo    [NQH*128, HID] bf16      w_o row slice
      cosT  [128, S]    f32  rope cos, transposed
      sinT  [128, S]    f32  rope sin, transposed, rows 0..63 negated
      dmask [128, QT//128, QT] bf16  causal masks for diagonal tiles
      ones  [128, 128]  bf16 all-ones (softmax denominator matmul)
    out: [S, HID] f32 partial output
    """
    from concourse import mybir
    from concourse.masks import make_identity

    nc = tc.nc
    f32, bf16 = mybir.dt.float32, mybir.dt.bfloat16
    Exp = mybir.ActivationFunctionType.Exp

    MT = NQH + 2          # qkv out tiles of 128 cols: NQH q heads, k, v
    KH = HID // P         # contraction chunks for qkv proj
    STn = S // SEQT       # seq tiles in qkv phase
    KCn = S // P          # key chunks in attention
    QTn = S // QT         # q tiles in attention
    RD = QT // P          # diagonal offsets per q tile
    NTn = HID // NT       # oproj col tiles
    KD = NQH              # oproj contraction chunks (d-chunks = q heads)
    qk_scale = 1.0 / math.sqrt(D)

    hsT, wq, wo, cosT_d, sinT_d, dmask_d, ones_d = (
        ins["hsT"], ins["wq"], ins["wo"], ins["cosT"], ins["sinT"],
        ins["dmask"], ins["ones"],
    )

    import contextlib

    with contextlib.ExitStack() as ctx:
        keep = ctx.enter_context(tc.tile_pool(name="keep", bufs=1))
        expp = ctx.enter_context(tc.tile_pool(name="expp", bufs=2 * KCn + 4))
        rcp = ctx.enter_context(tc.tile_pool(name="rcp", bufs=2))
        outp = ctx.enter_context(tc.tile_pool(name="outp", bufs=4))
        psum = ctx.enter_context(tc.tile_pool(name="psum", bufs=8, space="PSUM"))

        # persistent stores
        qk_store = [keep.tile([P, S], bf16, tag=f"qk{i}", name=f"qk{i}")
                    for i in range(NQH + 1)]
        v_store = keep.tile([P, KCn, D], bf16, tag="vst")
        attn_store = [keep.tile([P, S], bf16, tag=f"at{i}", name=f"at{i}")
                      for i in range(NQH)]
        dmask = keep.tile([P, RD, QT], bf16, tag="dmask")
        ones = keep.tile([P, P], bf16, tag="ones")
        ident = keep.tile([P, P], bf16, tag="ident")

        nc.sync.dma_start(out=dmask[:], in_=dmask_d[:])
        nc.sync.dma_start(out=ones[:], in_=ones_d[:])
        make_identity(nc, ident[:])

        # ---- phase 1: QKV projection (transposed out) + RoPE ----
        with tc.tile_pool(name="qkvp", bufs=1) as qkvp, \
             tc.tile_pool(name="hsp", bufs=3) as hsp, \
             tc.tile_pool(name="rope", bufs=2) as rope:
            cosT = qkvp.tile([P, S], f32, tag="cos")
            sinT = qkvp.tile([P, S], f32, tag="sin")
            nc.sync.dma_start(out=cosT[:], in_=cosT_d[:])
            nc.sync.dma_start(out=sinT[:], in_=sinT_d[:])

            wq3 = wq.rearrange("(kc p) m -> p kc m", p=P)  # [P, KH, MT*P]
            wq_sb = qkvp.tile([P, KH, MT * P], bf16, tag="wq")
            for kc in range(KH):
                nc.sync.dma_start(out=wq_sb[:, kc, :], in_=wq3[:, kc, :])

            for st in range(STn):
                cs = slice(st * SEQT, (st + 1) * SEQT)
                ps_tiles = [
                    psum.tile([P, SEQT], f32, tag="ps", name=f"qkvps{_i}")
                    for _i in range(MT)
                ]
                for kc in range(KH):
                    hs_t = hsp.tile([P, SEQT], bf16, tag="hs")
                    nc.sync.dma_start(
                        out=hs_t[:], in_=hsT[kc * P:(kc + 1) * P, cs]
                    )
                    for mt in range(MT):
                        nc.tensor.matmul(
                            ps_tiles[mt][:],
                            wq_sb[:, kc, mt * P:(mt + 1) * P],
                            hs_t[:],
                            start=(kc == 0),
                            stop=(kc == KH - 1),
                        )
                for mt in range(MT):
                    if mt < NQH + 1:
                        # rope: out = x*cos + swap_halves(x)*sin'  (f32 math)
                        tmp = rope.tile([P, SEQT], f32, tag="tmp")
                        nc.scalar.copy(tmp[:], ps_tiles[mt][:])
                        m1 = rope.tile([P, SEQT], f32, tag="m1")
                        nc.vector.tensor_mul(m1[:], ps_tiles[mt][:], cosT[:, cs])
                        sw = rope.tile([P, SEQT], f32, tag="sw")
                        nc.sync.dma_start(out=sw[0:64, :], in_=tmp[64:128, :])
                        nc.sync.dma_start(out=sw[64:128, :], in_=tmp[0:64, :])
                        m2 = rope.tile([P, SEQT], f32, tag="m2")
                        nc.vector.tensor_mul(m2[:], sw[:], sinT[:, cs])
                        nc.vector.tensor_add(qk_store[mt][:, cs], m1[:], m2[:])
                    else:
                        # V: transpose 128-blocks into [k, d] layout
                        vt = rope.tile([P, SEQT], bf16, tag="vt")
                        nc.scalar.copy(vt[:], ps_tiles[mt][:])
                        for j in range(SEQT // P):
                            tp = psum.tile([P, P], bf16, tag="ps", name="tp")
                            nc.tensor.transpose(
                                tp[:], vt[:, j * P:(j + 1) * P], ident[:]
                            )
                            nc.vector.tensor_copy(
                                v_store[:, st * (SEQT // P) + j, :], tp[:]
                            )

        # ---- phase 2/3: attention (scoresT -> exp -> PV + denom) and o_proj ----
        with tc.tile_pool(name="wop", bufs=1) as wop, \
             tc.tile_pool(name="astr", bufs=3) as astr:
            wo_sb = wop.tile([P, KD, HID], bf16, tag="wo")
            wo3 = wo.rearrange("(kd p) n -> p kd n", p=P)
            for kd in range(KD):
                nc.sync.dma_start(out=wo_sb[:, kd, :], in_=wo3[:, kd, :])

            for qt in range(QTn):
                qs = slice(qt * QT, (qt + 1) * QT)
                for h in range(NQH):
                    nkb = RD * qt + RD
                    # stream all score matmuls; exp trails on ScalarE
                    exs = []
                    for kb in range(nkb):
                        sp = psum.tile([P, QT], f32, tag="ps", name="sp")
                        nc.tensor.matmul(
                            sp[:],
                            qk_store[NQH][:, kb * P:(kb + 1) * P],
                            qk_store[h][:, qs],
                            start=True,
                            stop=True,
                        )
                        ex = expp.tile([P, QT], bf16, tag="ex", name="ex")
                        nc.scalar.activation(
                            out=ex[:], in_=sp[:], func=Exp, scale=qk_scale
                        )
                        r = kb - RD * qt
                        if r >= 0:
                            nc.vector.tensor_mul(ex[:], ex[:], dmask[:, r, :])
                        exs.append(ex)
                    ao = psum.tile([P, QT], f32, tag="ps", name="ao")
                    dn = psum.tile([P, QT], f32, tag="ps", name="dn")
                    for kb in range(nkb):
                        nc.tensor.matmul(
                            ao[:], v_store[:, kb, :], exs[kb][:],
                            start=(kb == 0), stop=(kb == nkb - 1),
                        )
                        nc.tensor.matmul(
                            dn[:], ones[:], exs[kb][:],
                            start=(kb == 0), stop=(kb == nkb - 1),
                        )
                    rc = rcp.tile([P, QT], f32, tag="rc")
                    nc.vector.reciprocal(rc[:], dn[:])
                    nc.vector.tensor_mul(attn_store[h][:, qs], ao[:], rc[:])

                # o_proj for the s-tiles covered by this q tile
                for st in range(qt * (QT // P), (qt + 1) * (QT // P)):
                    ss = slice(st * P, (st + 1) * P)
                    for nt in range(NTn):
                        po = psum.tile([P, NT], f32, tag="ps", name="po")
                        for kd in range(KD):
                            nc.tensor.matmul(
                                po[:],
                                attn_store[kd][:, ss],
                                wo_sb[:, kd, nt * NT:(nt + 1) * NT],
                                start=(kd == 0),
                                stop=(kd == KD - 1),
                            )
                        ot = outp.tile([P, NT], f32, tag="ot")
                        nc.scalar.copy(ot[:], po[:])
                        nc.sync.dma_start(
                            out=out[ss, nt * NT:(nt + 1) * NT], in_=ot[:]
                        )


def _host_inputs(hidden_states, w_qkv, w_o, position_ids):
    """Shard/transform full inputs into per-core input maps."""
    import ml_dtypes

    bf = ml_dtypes.bfloat16
    NQH = CFG["NQH"]
    QT = CFG["QT"]
    hs = np.asarray(hidden_states, dtype=np.float32).reshape(S, HID)
    hsT = np.ascontiguousarray(hs.T.astype(bf))  # [HID, S] bf16

    wqkv = np.asarray(w_qkv, dtype=np.float32)
    wo_full = np.asarray(w_o, dtype=np.float32)

    pos = np.asarray(position_ids).reshape(-1).astype(np.float32)  # [S]
    inv_freq = (
        1.0 / (ROPE_THETA ** (np.arange(0, D, 2, dtype=np.float32) / D))
    ).astype(np.float32)  # [D/2]
    freqs = pos[:, None] * inv_freq[None, :]  # [S, D/2]
    emb = np.concatenate([freqs, freqs], axis=1)  # [S, D]
    cosT = np.ascontiguousarray(np.cos(emb).T.astype(np.float32))  # [D, S]
    sinT = np.ascontiguousarray(np.sin(emb).T.astype(np.float32))
    sinT[: D // 2] *= -1.0  # fold rotate_half sign into sin

    RD = QT // P
    kk = np.arange(P)[:, None]
    jq = np.arange(QT)[None, :]
    dmask = np.ascontiguousarray(np.stack(
        [(r * P + kk <= jq) for r in range(RD)], axis=1
    ).astype(bf))  # [P, RD, QT]
    ones = np.ones((P, P), dtype=bf)

    in_maps = []
    for c in range(NCORES):
        qcols = wqkv[:, c * NQH * D:(c + 1) * NQH * D]
        kcols = wqkv[:, NH * D + c * D: NH * D + (c + 1) * D]
        vcols = wqkv[:, (NH + NKV) * D + c * D: (NH + NKV) * D + (c + 1) * D]
        wq_c = np.ascontiguousarray(
            np.concatenate([qcols, kcols, vcols], axis=1).astype(bf)
        )
        wo_c = np.ascontiguousarray(
            wo_full[c * NQH * D:(c + 1) * NQH * D, :].astype(bf)
        )
        in_maps.append(
            dict(hsT=hsT, wq=wq_c, wo=wo_c, cosT=cosT, sinT=sinT,
                 dmask=dmask, ones=ones)
        )
    return in_maps


_NC_CACHE = []


def _build_nc():
    import concourse.tile as tile
    from concourse import bacc, mybir

    f32, bf16 = mybir.dt.float32, mybir.dt.bfloat16
    NQH, QT = CFG["NQH"], CFG["QT"]
    nc = bacc.Bacc("TRN2", target_bir_lowering=False, debug=False,
                   num_devices=NCORES)
    ins = {
        "hsT": nc.dram_tensor("hsT", [HID, S], bf16, kind="ExternalInput").ap(),
        "wq": nc.dram_tensor("wq", [HID, (NQH + 2) * P], bf16,
                             kind="ExternalInput").ap(),
        "wo": nc.dram_tensor("wo", [NQH * P, HID], bf16,
                             kind="ExternalInput").ap(),
        "cosT": nc.dram_tensor("cosT", [P, S], f32, kind="ExternalInput").ap(),
        "sinT": nc.dram_tensor("sinT", [P, S], f32, kind="ExternalInput").ap(),
        "dmask": nc.dram_tensor("dmask", [P, QT // P, QT], bf16,
                                kind="ExternalInput").ap(),
        "ones": nc.dram_tensor("ones", [P, P], bf16, kind="ExternalInput").ap(),
    }
    out = nc.dram_tensor("out", [S, HID], f32, kind="ExternalOutput").ap()
    with tile.TileContext(nc) as tc:
        build_tile_kernel(tc, ins, out, **CFG)
    nc.compile()
    return nc


def kernel(hidden_states, w_qkv, w_o, attention_mask, position_ids):
    global LAST_EXEC_NS
    from concourse.bass_utils import run_bass_kernel_spmd

    if not _NC_CACHE:
        _NC_CACHE.append(_build_nc())
    nc = _NC_CACHE[0]

    in_maps = _host_inputs(hidden_states, w_qkv, w_o, position_ids)
    res = run_bass_kernel_spmd(nc, in_maps, list(range(NCORES)), trace=TRACE)
    LAST_EXEC_NS = res.exec_time_ns

    acc = np.zeros((S, HID), dtype=np.float32)
    for c in range(NCORES):
        acc += res.results[c]["out"]
    return acc.reshape(B, S, HID)


# revision 5
# speedup vs baseline: 1.6217x; 1.0588x over previous
"""Trainium2 Bass kernel: GQA attention block (QKV proj + RoPE + causal attention + o_proj).

Tensor-parallel over heads across 8 NeuronCores. Each core computes 4 Q heads /
1 KV head and a partial o_proj output; the host sums the 8 partials.

Compute: bf16 matmuls (fp32 PSUM accumulate), fp32 softmax, bf16 storage.
Pipelined per 512-row block: QKV(blk) -> attention(qt=blk) -> o_proj(blk).

Self-contained: hardcodes problem shapes from the spec
  hidden_states [1, 2048, 4096], w_qkv [4096, 6144], w_o [4096, 4096],
  attention_mask causal tril [1,1,2048,2048], position_ids arange [1, 2048].
"""

import math
import sys

sys.path.insert(0, "/opt/trn_rl_repo")

import numpy as np

# problem shapes
B, S, HID = 1, 2048, 4096
NH, NKV, D = 32, 8, 128
ROPE_THETA = 10000.0
NCORES = 8
P = 128

# per-core tiling config
CFG = dict(S=S, HID=HID, NQH=NH // NCORES, SEQT=512, QT=512, NT=512)

# set by test harness to capture HW exec time via NTFF trace
TRACE = False
LAST_EXEC_NS = None


def build_tile_kernel(tc, ins, out, S, HID, NQH, SEQT, QT, NT):
    """Emit the per-core attention kernel into TileContext `tc`.

    ins: dict of DRAM APs (bf16 unless noted):
      hsT   [HID, S]      hidden_states transposed (replicated)
      wq    [HID, (NQH+2)*128]  w_qkv column slice (Q heads | K | V)
      wo    [NQH*128, HID]      w_o row slice
      cosT  [128, S]      rope cos, transposed
      sinT  [128, S]      rope sin, transposed, rows 0..63 negated
      dmask [128, QT//128, QT]  causal masks for diagonal tiles
      ones  [128, 128]    all-ones (softmax denominator matmul)
    out: [S, HID] bf16 partial output
    """
    from concourse import mybir
    from concourse.masks import make_identity

    nc = tc.nc
    f32, bf16 = mybir.dt.float32, mybir.dt.bfloat16
    Exp = mybir.ActivationFunctionType.Exp

    assert SEQT == QT, "block pipeline assumes one q-tile per seq tile"
    MT = NQH + 2          # qkv out tiles of 128 cols: NQH q heads, k, v
    KH = HID // P         # contraction chunks for qkv proj
    STn = S // SEQT       # seq/q blocks
    KCn = S // P          # key chunks in attention
    RD = QT // P          # diagonal offsets per q tile
    NTn = HID // NT       # oproj col tiles
    KD = NQH              # oproj contraction chunks (d-chunks = q heads)
    qk_scale = 1.0 / math.sqrt(D)

    hsT, wq, wo, cosT_d, sinT_d, dmask_d, ones_d = (
        ins["hsT"], ins["wq"], ins["wo"], ins["cosT"], ins["sinT"],
        ins["dmask"], ins["ones"],
    )
    wq3 = wq.rearrange("(kc p) m -> p kc m", p=P)  # [P, KH, MT*P]
    wo3 = wo.rearrange("(kd p) n -> p kd n", p=P)  # [P, KD, HID]

    import contextlib

    with contextlib.ExitStack() as ctx:
        keep = ctx.enter_context(tc.tile_pool(name="keep", bufs=1))
        hsp = ctx.enter_context(tc.tile_pool(name="hsp", bufs=3))
        rope = ctx.enter_context(tc.tile_pool(name="rope", bufs=2))
        expp = ctx.enter_context(tc.tile_pool(name="expp", bufs=KCn + 4))
        rcp = ctx.enter_context(tc.tile_pool(name="rcp", bufs=2))
        outp = ctx.enter_context(tc.tile_pool(name="outp", bufs=4))
        psum = ctx.enter_context(tc.tile_pool(name="psum", bufs=8, space="PSUM"))

        # persistent stores
        qk_store = [keep.tile([P, S], bf16, tag=f"qk{i}", name=f"qk{i}")
                    for i in range(NQH + 1)]
        v_store = keep.tile([P, KCn, D], bf16, tag="vst")
        attn_store = [keep.tile([P, S], bf16, tag=f"at{i}", name=f"at{i}")
                      for i in range(NQH)]
        dmask = keep.tile([P, RD, QT], bf16, tag="dmask")
        ones = keep.tile([P, P], bf16, tag="ones")
        ident = keep.tile([P, P], bf16, tag="ident")
        cosT = keep.tile([P, S], bf16, tag="cos")
        sinT = keep.tile([P, S], bf16, tag="sin")
        wq_sb = keep.tile([P, KH, MT * P], bf16, tag="wq")
        wo_sb = keep.tile([P, KD, HID], bf16, tag="wo")

        def emit_qkv(blk):
            cs = slice(blk * SEQT, (blk + 1) * SEQT)
            ps_tiles = [
                psum.tile([P, SEQT], f32, tag="ps", name=f"qkvps{_i}")
                for _i in range(MT)
            ]
            for kc in range(KH):
                if blk == 0:
                    # interleave weight/activation loads so MMs start early
                    nc.sync.dma_start(out=wq_sb[:, kc, :], in_=wq3[:, kc, :])
                hs_t = hsp.tile([P, SEQT], bf16, tag="hs")
                nc.sync.dma_start(out=hs_t[:], in_=hsT[kc * P:(kc + 1) * P, cs])
                for mt in range(MT):
                    nc.tensor.matmul(
                        ps_tiles[mt][:],
                        wq_sb[:, kc, mt * P:(mt + 1) * P],
                        hs_t[:],
                        start=(kc == 0),
                        stop=(kc == KH - 1),
                    )
            if blk == 0:
                nc.sync.dma_start(out=cosT[:], in_=cosT_d[:])
                nc.sync.dma_start(out=sinT[:], in_=sinT_d[:])
                nc.sync.dma_start(out=dmask[:], in_=dmask_d[:])
                nc.sync.dma_start(out=ones[:], in_=ones_d[:])
                make_identity(nc, ident[:])
            for mt in range(MT):
                if mt < NQH + 1:
                    # rope: out = x*cos + swap_halves(x)*sin'  (f32 x bf16)
                    tmp = rope.tile([P, SEQT], f32, tag="tmp")
                    nc.scalar.copy(tmp[:], ps_tiles[mt][:])
                    m1 = rope.tile([P, SEQT], f32, tag="m1")
                    nc.vector.tensor_mul(m1[:], ps_tiles[mt][:], cosT[:, cs])
                    sw = rope.tile([P, SEQT], f32, tag="sw")
                    nc.sync.dma_start(out=sw[0:64, :], in_=tmp[64:128, :])
                    nc.sync.dma_start(out=sw[64:128, :], in_=tmp[0:64, :])
                    m2 = rope.tile([P, SEQT], f32, tag="m2")
                    nc.vector.tensor_mul(m2[:], sw[:], sinT[:, cs])
                    nc.vector.tensor_add(qk_store[mt][:, cs], m1[:], m2[:])
                else:
                    # V: transpose 128-blocks into [k, d] layout
                    vt = rope.tile([P, SEQT], bf16, tag="vt")
                    nc.scalar.copy(vt[:], ps_tiles[mt][:])
                    for j in range(SEQT // P):
                        tp = psum.tile([P, P], bf16, tag="ps", name="tp")
                        nc.tensor.transpose(
                            tp[:], vt[:, j * P:(j + 1) * P], ident[:]
                        )
                        nc.vector.tensor_copy(
                            v_store[:, blk * (SEQT // P) + j, :], tp[:]
                        )

        def emit_attention(qt):
            qs = slice(qt * QT, (qt + 1) * QT)
            for h in range(NQH):
                nkb = RD * qt + RD
                # stream all score matmuls; exp trails on ScalarE
                exs = []
                for kb in range(nkb):
                    sp = psum.tile([P, QT], f32, tag="ps", name="sp")
                    nc.tensor.matmul(
                        sp[:],
                        qk_store[NQH][:, kb * P:(kb + 1) * P],
                        qk_store[h][:, qs],
                        start=True,
                        stop=True,
                    )
                    ex = expp.tile([P, QT], bf16, tag="ex", name="ex")
                    nc.scalar.activation(
                        out=ex[:], in_=sp[:], func=Exp, scale=qk_scale
                    )
                    r = kb - RD * qt
                    if r >= 0:
                        nc.vector.tensor_mul(ex[:], ex[:], dmask[:, r, :])
                    exs.append(ex)
                ao = psum.tile([P, QT], f32, tag="ps", name="ao")
                dn = psum.tile([P, QT], f32, tag="ps", name="dn")
                for kb in range(nkb):
                    nc.tensor.matmul(
                        ao[:], v_store[:, kb, :], exs[kb][:],
                        start=(kb == 0), stop=(kb == nkb - 1),
                    )
                    nc.tensor.matmul(
                        dn[:], ones[:], exs[kb][:],
                        start=(kb == 0), stop=(kb == nkb - 1),
                    )
                rc = rcp.tile([P, QT], f32, tag="rc")
                nc.vector.reciprocal(rc[:], dn[:])
                nc.vector.tensor_mul(attn_store[h][:, qs], ao[:], rc[:])

        def emit_oproj(blk):
            if blk == 0:
                for kd in range(KD):
                    nc.sync.dma_start(out=wo_sb[:, kd, :], in_=wo3[:, kd, :])
            for st in range(blk * (QT // P), (blk + 1) * (QT // P)):
                ss = slice(st * P, (st + 1) * P)
                for nt in range(NTn):
                    po = psum.tile([P, NT], f32, tag="ps", name="po")
                    for kd in range(KD):
                        nc.tensor.matmul(
                            po[:],
                            attn_store[kd][:, ss],
                            wo_sb[:, kd, nt * NT:(nt + 1) * NT],
                            start=(kd == 0),
                            stop=(kd == KD - 1),
                        )
                    ot = outp.tile([P, NT], bf16, tag="ot")
                    if nt % 2 == 0:
                        nc.scalar.copy(ot[:], po[:])
                    else:
                        nc.vector.tensor_copy(ot[:], po[:])
                    nc.sync.dma_start(
                        out=out[ss, nt * NT:(nt + 1) * NT], in_=ot[:]
                    )

        for blk in range(STn):
            emit_qkv(blk)
            emit_attention(blk)
            emit_oproj(blk)


def _host_inputs(hidden_states, w_qkv, w_o, position_ids):
    """Shard/transform full inputs into per-core input maps."""
    import ml_dtypes

    bf = ml_dtypes.bfloat16
    NQH = CFG["NQH"]
    QT = CFG["QT"]
    hs = np.asarray(hidden_states, dtype=np.float32).reshape(S, HID)
    hsT = np.ascontiguousarray(hs.T.astype(bf))  # [HID, S] bf16

    wqkv = np.asarray(w_qkv, dtype=np.float32)
    wo_full = np.asarray(w_o, dtype=np.float32)

    pos = np.asarray(position_ids).reshape(-1).astype(np.float32)  # [S]
    inv_freq = (
        1.0 / (ROPE_THETA ** (np.arange(0, D, 2, dtype=np.float32) / D))
    ).astype(np.float32)  # [D/2]
    freqs = pos[:, None] * inv_freq[None, :]  # [S, D/2]
    emb = np.concatenate([freqs, freqs], axis=1)  # [S, D]
    cosT = np.ascontiguousarray(np.cos(emb).T.astype(bf))  # [D, S]
    sinT_f = np.sin(emb).T.astype(np.float32)
    sinT_f[: D // 2] *= -1.0  # fold rotate_half sign into sin
    sinT = np.ascontiguousarray(sinT_f.astype(bf))

    RD = QT // P
    kk = np.arange(P)[:, None]
    jq = np.arange(QT)[None, :]
    dmask = np.ascontiguousarray(np.stack(
        [(r * P + kk <= jq) for r in range(RD)], axis=1
    ).astype(bf))  # [P, RD, QT]
    ones = np.ones((P, P), dtype=bf)

    in_maps = []
    for c in range(NCORES):
        qcols = wqkv[:, c * NQH * D:(c + 1) * NQH * D]
        kcols = wqkv[:, NH * D + c * D: NH * D + (c + 1) * D]
        vcols = wqkv[:, (NH + NKV) * D + c * D: (NH + NKV) * D + (c + 1) * D]
        wq_c = np.ascontiguousarray(
            np.concatenate([qcols, kcols, vcols], axis=1).astype(bf)
        )
        wo_c = np.ascontiguousarray(
            wo_full[c * NQH * D:(c + 1) * NQH * D, :].astype(bf)
        )
        in_maps.append(
            dict(hsT=hsT, wq=wq_c, wo=wo_c, cosT=cosT, sinT=sinT,
                 dmask=dmask, ones=ones)
        )
    return in_maps


_NC_CACHE = []


def _build_nc():
    import concourse.tile as tile
    from concourse import bacc, mybir

    bf16 = mybir.dt.bfloat16
    NQH, QT = CFG["NQH"], CFG["QT"]
    nc = bacc.Bacc("TRN2", target_bir_lowering=False, debug=False,
                   num_devices=NCORES)
    ins = {
        "hsT": nc.dram_tensor("hsT", [HID, S], bf16, kind="ExternalInput").ap(),
        "wq": nc.dram_tensor("wq", [HID, (NQH + 2) * P], bf16,
                             kind="ExternalInput").ap(),
        "wo": nc.dram_tensor("wo", [NQH * P, HID], bf16,
                             kind="ExternalInput").ap(),
        "cosT": nc.dram_tensor("cosT", [P, S], bf16, kind="ExternalInput").ap(),
        "sinT": nc.dram_tensor("sinT", [P, S], bf16, kind="ExternalInput").ap(),
        "dmask": nc.dram_tensor("dmask", [P, QT // P, QT], bf16,
                                kind="ExternalInput").ap(),
        "ones": nc.dram_tensor("ones", [P, P], bf16, kind="ExternalInput").ap(),
    }
    out = nc.dram_tensor("out", [S, HID], bf16, kind="ExternalOutput").ap()
    with tile.TileContext(nc) as tc:
        build_tile_kernel(tc, ins, out, **CFG)
    nc.compile()
    return nc


def kernel(hidden_states, w_qkv, w_o, attention_mask, position_ids):
    global LAST_EXEC_NS
    from concourse.bass_utils import run_bass_kernel_spmd

    if not _NC_CACHE:
        _NC_CACHE.append(_build_nc())
    nc = _NC_CACHE[0]

    in_maps = _host_inputs(hidden_states, w_qkv, w_o, position_ids)
    res = run_bass_kernel_spmd(nc, in_maps, list(range(NCORES)), trace=TRACE)
    LAST_EXEC_NS = res.exec_time_ns

    acc = np.zeros((S, HID), dtype=np.float32)
    for c in range(NCORES):
        acc += res.results[c]["out"].astype(np.float32)
    return acc.reshape(B, S, HID)
